# revision 1
# baseline (speedup 1.0000x reference)
"""Trainium2 Bass kernel for EntityAttentionLayer — bf16 redesign.

entities[4096,128,256] -> fused QKV (W_in [1536,256]) -> 4-head attention
(queries = first 32 entities, pre_mask True => -inf logits) -> out proj
(W_out [512,512] + b_out) -> post_mask True => 0.

Data-parallel over batch across 8 NeuronCores (512 batch elems per core).

Key design points vs the f32 baseline:
- All matmuls in bf16 (1 PE cycle/row vs 4 for f32; fp8 is too lossy for
  the 2e-2 gate: per-stage error is ~eps*sqrt(2), no 1/sqrt(K) averaging).
- Entities pre-transposed/pre-cast on the host to bf16 [g8, kc, i, b, e]
  so the contraction dim lands on partitions straight from DMA — no PE
  transposes, no PSUM round-trip, half the HBM bytes.
- Logits via col-tiled M=32 matmuls at partition offset 32h per head.
- Softmax: exp with fused scale, no max-subtraction; post-mask folded into
  the normalization; output tail is a single fused DVE op
  (bias*notpost + psum) since masked rows of the projection are already 0.
- Software-pipelined with stage skew: attention tails of group g8-1 run
  between the K and V projections of group g8, interleaved across the two
  4-batch halves so PSUM->SBUF copies hide behind PE matmul work while the
  softmax chain of g8 runs on ACT/DVE.
- DMAs spread across queues (entities on SP, masks on gpsimd, stores on
  ACT) so no single sequencer serializes.
"""

import math
import sys

import numpy as np

for _p in ("/opt/trn_rl_repo", "/root/.axon_site/_ro/trn_rl_repo"):
    if _p not in sys.path:
        sys.path.insert(0, _p)

import ml_dtypes

import concourse.bass as bass
import concourse.tile as tile
from concourse import bacc, mybir
from concourse.bass_utils import run_bass_kernel_spmd
from concourse.masks import make_identity

F32 = mybir.dt.float32
BF16 = mybir.dt.bfloat16
NP_BF16 = ml_dtypes.bfloat16

N_CORES = 8
B = 512           # batch elems per core
NE = 128          # entities
NQ = 32           # queries
IN = 256          # input dim
E = 512           # embed dim
H = 4             # heads
HD = 128          # head dim
OUT = 512         # out dim
SCALE = 1.0 / math.sqrt(HD)
NEG_BIG = -1.0e30


class _Balancer:
    """Round-robin PSUM->SBUF copies between scalar (ACT) and vector (DVE)
    engines weighted by their modeled busy-ns."""

    def __init__(self, nc):
        self.nc = nc
        self.act = 0.0
        self.dve = 0.0

    def copy(self, out, in_):
        fd = 1
        for s in in_.shape[1:]:
            fd *= s
        act_cost = (222.0 + fd) / 1.2
        dve_cost = (120.0 + fd) / 0.96
        if self.act + act_cost <= self.dve + dve_cost:
            self.act += act_cost
            self.nc.scalar.copy(out=out, in_=in_)
        else:
            self.dve += dve_cost
            self.nc.vector.tensor_copy(out=out, in_=in_)

    def charge_act(self, ns):
        self.act += ns

    def charge_dve(self, ns):
        self.dve += ns


def _build(b_per_core=B):
    nb = b_per_core
    assert nb % 8 == 0
    n_g8 = nb // 8
    n_g4 = nb // 4
    nc = bacc.Bacc(None, target_bir_lowering=False, debug=False)

    # host-prepped inputs
    entT_d = nc.dram_tensor("entT", [n_g8, 2, 128, 8, NE], BF16,
                            kind="ExternalInput").ap()
    maskb_d = nc.dram_tensor("maskb", [nb, NQ, NE], mybir.dt.uint8,
                             kind="ExternalInput").ap()
    winT_d = nc.dram_tensor("w_inT", [IN, 3 * E], BF16, kind="ExternalInput").ap()
    woutT_d = nc.dram_tensor("w_outT", [E, OUT], BF16, kind="ExternalInput").ap()
    npcolT_d = nc.dram_tensor("npcolT", [128, nb], F32, kind="ExternalInput").ap()
    npoT_d = nc.dram_tensor("npoT", [128, n_g4], F32, kind="ExternalInput").ap()
    bout_d = nc.dram_tensor("b_out", [OUT], F32, kind="ExternalInput").ap()
    out_d = nc.dram_tensor("out", [nb, NQ, OUT], BF16, kind="ExternalOutput").ap()

    with tile.TileContext(nc) as tc:
        with (
            tc.tile_pool(name="const", bufs=1) as cpool,
            tc.tile_pool(name="ent", bufs=3) as p_ent,
            tc.tile_pool(name="qkv", bufs=3) as p_qkv,
            tc.tile_pool(name="attn", bufs=4) as p_attn,
            tc.tile_pool(name="outb", bufs=3) as p_out,
            tc.tile_pool(name="psj", bufs=5, space="PSUM") as p_ps,
            tc.tile_pool(name="psl", bufs=2, space="PSUM") as p_psl,
            tc.tile_pool(name="pst", bufs=1, space="PSUM") as p_pst,
        ):
            bal = _Balancer(nc)

            # ---- constants ----
            winT_sb = cpool.tile([128, 2, 3 * E], BF16)
            for kc in range(2):
                nc.sync.dma_start(
                    out=winT_sb[:, kc, :], in_=winT_d[kc * 128:(kc + 1) * 128, :]
                )
            woutT_sb = cpool.tile([128, 4, OUT], BF16)
            for ec in range(4):
                nc.sync.dma_start(
                    out=woutT_sb[:, ec, :], in_=woutT_d[ec * 128:(ec + 1) * 128, :]
                )
            npcolT_sb = cpool.tile([128, nb], F32)
            nc.sync.dma_start(out=npcolT_sb, in_=npcolT_d)
            npoT_sb = cpool.tile([128, n_g4], F32)
            nc.sync.dma_start(out=npoT_sb, in_=npoT_d)
            bias_rep = cpool.tile([128, OUT], F32)
            nc.sync.dma_start(
                out=bias_rep,
                in_=bass.AP(tensor=bout_d.tensor, offset=bout_d.offset,
                            ap=[[0, 128], [1, OUT]]),
            )
            ident_bf = cpool.tile([128, 128], BF16)
            make_identity(nc, ident_bf)
            negbig = cpool.tile([128, 4, NE], F32)
            nc.gpsimd.memset(negbig, NEG_BIG)

            # ---- software-pipelined main loop over groups of 8 ----
            # Stage A(g8): DMA + Q/K/V projections + logits + softmax chain
            # (the chain runs on ACT/DVE). Stage B(g8): prob transpose,
            # attn@V, out-proj, store. B(g8-1) is emitted after A(g8)'s
            # logits so the PE has projection work to chew on while the
            # softmax chain of g8 runs on the other engines.

            def stage_a(g8, tail_state):
                ent8 = p_ent.tile([128, 2, 8, NE], BF16, tag="ent8")
                nc.sync.dma_start(
                    out=ent8, in_=entT_d[g8].rearrange("k p b e -> p k b e")
                )
                m8 = p_attn.tile([128, 8, NE], mybir.dt.uint8, tag="m8")
                for h in range(H):
                    nc.gpsimd.dma_start(
                        out=m8[h * NQ:(h + 1) * NQ, :, :],
                        in_=maskb_d[8 * g8:8 * g8 + 8].rearrange(
                            "b q e -> q b e"),
                    )

                # Q^T: qT[:, mc, b, q]
                qT = p_qkv.tile([128, 4, 8, NQ], BF16, tag="qT")
                for jh in range(2):
                    ps_q = p_ps.tile([128, 2, 8, NQ], F32, tag="ps")
                    for mh in range(2):
                        mc = jh * 2 + mh
                        for kc in range(2):
                            nc.tensor.matmul(
                                ps_q[:, mh].rearrange("p b q -> p (b q)"),
                                winT_sb[:, kc, mc * 128:(mc + 1) * 128],
                                ent8[:, kc, :, 0:NQ],
                                start=(kc == 0), stop=(kc == 1),
                            )
                    bal.copy(
                        out=qT[:, jh * 2:(jh + 1) * 2].rearrange(
                            "p m b q -> p (m b q)"),
                        in_=ps_q.rearrange("p m b q -> p (m b q)"),
                    )

                # K^T: k8[:, mc, b, e]
                k8 = p_qkv.tile([128, H, 8, NE], BF16, tag="k8")
                for mc in range(H):
                    for bh in range(2):
                        ps_k = p_ps.tile([128, 4, NE], F32, tag="ps")
                        for kc in range(2):
                            nc.tensor.matmul(
                                ps_k.rearrange("p b e -> p (b e)"),
                                winT_sb[:, kc, E + mc * 128:E + (mc + 1) * 128],
                                ent8[:, kc, bh * 4:(bh + 1) * 4, :],
                                start=(kc == 0), stop=(kc == 1),
                            )
                        bal.copy(
                            out=k8[:, mc, bh * 4:(bh + 1) * 4, :].rearrange(
                                "p b e -> p (b e)"),
                            in_=ps_k.rearrange("p b e -> p (b e)"),
                        )

                if tail_state is not None:
                    stage_b(tail_state)

                # V natural layout: v8[:, j, :] = V [e, 512]
                v8 = p_qkv.tile([128, 8, E], BF16, tag="v8")
                for j in range(8):
                    ps_v = p_ps.tile([128, E], F32, tag="ps")
                    for kc in range(2):
                        nc.tensor.matmul(
                            ps_v,
                            ent8[:, kc, j, :],
                            winT_sb[:, kc, 2 * E:3 * E],
                            start=(kc == 0), stop=(kc == 1),
                        )
                    bal.copy(out=v8[:, j, :], in_=ps_v)

                # logits for both 4-batch halves
                ps_ls = []
                for g4h in range(2):
                    ps_l = p_psl.tile([128, 4, NE], F32, tag="psl")
                    for bl in range(4):
                        j = g4h * 4 + bl
                        for h in range(H):
                            nc.tensor.matmul(
                                ps_l[h * NQ:(h + 1) * NQ, bl, :],
                                qT[:, h, j, :],
                                k8[:, h, j, :],
                                start=True, stop=True,
                                tile_position=(0, h * NQ),
                            )
                    ps_ls.append(ps_l)
                return g8, m8, v8, ps_ls

            def stage_chain(state):
                """softmax chain (ACT/DVE): mask, exp, sums, normalize."""
                g8, m8, v8, ps_ls = state
                pns = []
                for g4h in range(2):
                    g = g8 * 2 + g4h
                    ps_l = ps_ls[g4h]
                    nc.vector.copy_predicated(
                        ps_l, m8[:, g4h * 4:(g4h + 1) * 4, :], negbig
                    )
                    bal.charge_dve((120 + 512) / 0.96)
                    pexp = p_attn.tile([128, 4, NE], BF16, tag="pexp")
                    for bh2 in range(2):
                        nc.scalar.activation(
                            out=pexp[:, 2 * bh2:2 * bh2 + 2, :].rearrange(
                                "p b e -> p (b e)"),
                            in_=ps_l[:, 2 * bh2:2 * bh2 + 2, :].rearrange(
                                "p b e -> p (b e)"),
                            func=mybir.ActivationFunctionType.Exp,
                            scale=SCALE,
                        )
                        bal.charge_act((222 + 256) / 1.2)
                    s4 = p_attn.tile([128, 4], F32, tag="s4")
                    nc.vector.tensor_reduce(
                        out=s4, in_=pexp, axis=mybir.AxisListType.X,
                        op=mybir.AluOpType.add,
                    )
                    bal.charge_dve((58 + 512) / 0.96)
                    r4 = p_attn.tile([128, 4], F32, tag="r4")
                    nc.vector.tensor_scalar_max(r4, s4, 1.0e-30)
                    nc.vector.reciprocal(r4, r4)
                    nc.vector.tensor_mul(r4, r4, npcolT_sb[:, 4 * g:4 * g + 4])
                    bal.charge_dve(200.0)
                    pn = p_attn.tile([128, 4, NE], BF16, tag="pn")
                    for bl in range(4):
                        nc.vector.tensor_scalar_mul(
                            pn[:, bl, :], pexp[:, bl, :], r4[:, bl:bl + 1]
                        )
                        bal.charge_dve((58 + 64) / 0.96)
                    pns.append(pn)
                return g8, v8, pns

            def stage_b(state):
                """attention tails, interleaved across the two 4-batch halves
                so each PSUM->SBUF copy hides behind the other half's PE
                work: transposes(0,1) -> attnV(0,1) -> out-proj(0,1)."""
                g8, v8, pns = state
                pt4s, ao4s = [], []
                for g4h in range(2):
                    pn = pns[g4h]
                    ps_pt = p_pst.tile([128, 4, 128], BF16, tag="pst")
                    for bl in range(4):
                        nc.tensor.transpose(
                            ps_pt[:, bl, :], pn[:, bl, :], ident_bf
                        )
                    pt4 = p_attn.tile([128, 4, 128], BF16, tag="pt4")
                    bal.copy(out=pt4.rearrange("p b x -> p (b x)"),
                             in_=ps_pt.rearrange("p b x -> p (b x)"))
                    pt4s.append(pt4)

                for g4h in range(2):
                    pt4 = pt4s[g4h]
                    ps_ao = p_ps.tile([128, H, 4, NQ], F32, tag="ps")
                    for bl in range(4):
                        j = g4h * 4 + bl
                        for h in range(H):
                            nc.tensor.matmul(
                                ps_ao[:, h, bl, :],
                                v8[:, j, h * 128:(h + 1) * 128],
                                pt4[:, bl, h * NQ:(h + 1) * NQ],
                                start=True, stop=True,
                            )
                    ao4 = p_attn.tile([128, H, 4, NQ], BF16, tag="ao4")
                    for hh in range(2):
                        bal.copy(
                            out=ao4[:, 2 * hh:2 * hh + 2].rearrange(
                                "p h b q -> p (h b q)"),
                            in_=ps_ao[:, 2 * hh:2 * hh + 2].rearrange(
                                "p h b q -> p (h b q)"),
                        )
                    ao4s.append(ao4)

                for g4h in range(2):
                    g = g8 * 2 + g4h
                    ao4 = ao4s[g4h]
                    ps_o = p_ps.tile([128, OUT], F32, tag="ps")
                    for ec in range(4):
                        nc.tensor.matmul(
                            ps_o,
                            ao4[:, ec, :, :].rearrange("p b q -> p (b q)"),
                            woutT_sb[:, ec, :],
                            start=(ec == 0), stop=(ec == 3),
                        )
                    # masked rows of ps_o are already 0 (probs were
                    # pre-zeroed), so only the bias needs the post-mask:
                    # of = bias*notpost + ps_o in one fused DVE op
                    of = p_out.tile([128, OUT], BF16, tag="of")
                    nc.vector.scalar_tensor_tensor(
                        out=of, in0=bias_rep, scalar=npoT_sb[:, g:g + 1],
                        in1=ps_o, op0=mybir.AluOpType.mult,
                        op1=mybir.AluOpType.add,
                    )
                    bal.charge_dve((120 + 512) / 0.96)
                    nc.scalar.dma_start(
                        out=out_d[4 * g:4 * g + 4].rearrange("b q n -> (b q) n"),
                        in_=of,
                    )

            prev = None
            for g8 in range(n_g8):
                st = stage_a(g8, prev)
                prev = stage_chain(st)
            stage_b(prev)

    nc.compile()
    return nc


_CACHE = {}


def _get_nc(nb):
    if nb not in _CACHE:
        _CACHE[nb] = _build(nb)
    return _CACHE[nb]


def _fast_bf16(x):
    """f32 -> bf16 with round-to-nearest-even via uint bit ops (the ml_dtypes
    .astype ufunc is ~100x slower)."""
    x = np.ascontiguousarray(x, dtype=np.float32)
    v = x.view(np.uint32)
    r = ((v + 0x7FFF + ((v >> 16) & 1)) >> 16).astype(np.uint16)
    return r.view(NP_BF16)


def _make_in_maps(inputs):
    entities = np.asarray(inputs["entities"], dtype=np.float32)
    pre = np.asarray(inputs["pre_mask"]).astype(bool)
    post = np.asarray(inputs["post_mask"]).astype(bool)
    w_in = np.asarray(inputs["W_in"], dtype=np.float32)
    w_out = np.asarray(inputs["W_out"], dtype=np.float32)
    bout = np.asarray(inputs["b_out"], dtype=np.float32)

    bs = entities.shape[0]
    nb = bs // N_CORES
    n_g8 = nb // 8
    n_g4 = nb // 4

    winT = _fast_bf16(np.ascontiguousarray(w_in.T))
    woutT = _fast_bf16(np.ascontiguousarray(w_out.T))

    # entT[g8, kc, i, b, e] = entities[8*g8+b, e, kc*128+i]  (per core slice);
    # cast to bf16 first so the transpose copy moves half the bytes
    entb = _fast_bf16(entities)
    entT_all = entb.reshape(bs // 8, 8, NE, 2, 128).transpose(0, 3, 4, 1, 2)
    entT_all = np.ascontiguousarray(entT_all)

    maskb_all = pre.astype(np.uint8)

    notpost = (~post).astype(np.float32)          # [bs, NQ]
    # npcolT[h*32+q, j] = notpost[j, q] (within core)
    # npr[g*128 + bl*32 + q] = notpost[4g+bl, q]
    in_maps = []
    for c in range(N_CORES):
        sl = slice(c * nb, (c + 1) * nb)
        np_c = notpost[sl]                         # [nb, NQ]
        npcolT = np.tile(np_c.T, (H, 1))           # [128, nb]
        # npoT[bl*32+q, g] = notpost[4g+bl, q]
        npoT = np_c.reshape(n_g4, 4 * NQ).T
        in_maps.append({
            "entT": np.ascontiguousarray(entT_all[c * n_g8:(c + 1) * n_g8]),
            "maskb": np.ascontiguousarray(maskb_all[sl]),
            "w_inT": winT,
            "w_outT": woutT,
            "b_out": bout,
            "npcolT": np.ascontiguousarray(npcolT),
            "npoT": np.ascontiguousarray(npoT),
        })
    return in_maps


def kernel(entities, pre_mask, post_mask, W_in, W_out, b_out):
    in_maps = _make_in_maps({
        "entities": entities, "pre_mask": pre_mask, "post_mask": post_mask,
        "W_in": W_in, "W_out": W_out, "b_out": b_out,
    })
    nb = in_maps[0]["maskb"].shape[0]
    nc = _get_nc(nb)
    res = run_bass_kernel_spmd(nc, in_maps, list(range(N_CORES)))
    out = np.concatenate(
        [np.asarray(res.results[c]["out"], dtype=np.float32)
         for c in range(N_CORES)],
        axis=0,
    )
    return out

